# revision 81
# baseline (speedup 1.0000x reference)
"""Trainium2 (Bass/Tile) 8-core kernel for a dense transformer block.

Math (mirrors the reference):
    q      = x @ wi                       # all heads share wi -> q == k == v
    P      = softmax(mask(q q^T / 32))
    head   = q + P @ q
    h      = head @ W_eff + bias          # cat of identical heads @ out_kernel
                                          # == head @ (sum of the 8 blocks)
    hn     = layernorm(h)                 # E[x^2]-E[x]^2 variance, eps=1e-5
    out    = silu(hn @ wi) @ wi

Sharding (8 NeuronCores, one SPMD NEFF):
    core c = (batch b=c//4, rank r=c%4).  Core (b,r) owns four 128-row
    q-tiles of batch b at rows 512*t + 128*r (slot t = 0..3).  Causal
    padding: the slot-t tile attends only key banks [0, 512*(t+1)) --
    uniform across cores, so one program serves all 8 (every per-core
    difference lives in input data, incl. a single 128x512 additive
    diagonal-mask tile that is the same for all four slots).

    q is computed directly in transposed layout (q^T: d on partitions) and
    AllGathered tile-by-tile (4 chunked AllGathers) within each 4-core
    batch group; key order in SBUF is slot-major so causal key ranges are
    contiguous.  q natural layout (for P@V) is rebuilt locally by DMA
    transpose.  W_eff comes from an 8-core AllReduce over the bf16
    out_kernel blocks, read directly from DRAM.  All inputs are host-cast
    to bf16.  LayerNorm rstd uses exp(-0.5*ln(var+eps)) so the entire
    attention+LN stretch stays in one Activation function set.
"""

import sys

for _p in ("/opt/trn_rl_repo",):
    if _p not in sys.path:
        sys.path.insert(0, _p)

import numpy as np

B, S, D, H = 2, 2048, 1024, 8
NCORES = 8
TOK = 512          # tokens (q rows) per core
NT = 4             # 128-row q tiles per core (one per causal slot)
MASK_NEG = -1.0e6  # pre-scale additive mask value (exp(-1e6/32) == 0)
EPS = 1e-5
CHUNK = 8 * 128 * 128   # bf16 elems per gather chunk (one rank's q^T tile)

_CACHE = {}


def _build(reps=1, sim_cc_as_dma=False, debug=False):
    import concourse.bacc as bacc
    import concourse.mybir as mybir
    import concourse.tile as tile
    from concourse.replica_groups import maybe_share_collective_output_space

    dt = mybir.dt
    BF, F32 = dt.bfloat16, dt.float32
    F8 = dt.float8e4
    DR = mybir.MatmulPerfMode.DoubleRow
    AF = mybir.ActivationFunctionType
    AX = mybir.AxisListType
    ALU = mybir.AluOpType

    nc = bacc.Bacc("TRN2", target_bir_lowering=False, debug=False,
                   num_devices=NCORES)

    # ---------------- I/O (per-core shapes, all bf16 via host cast) --------
    xt_d = nc.dram_tensor("xt", [NT * D, 128], BF, kind="ExternalInput")
    wi_d = nc.dram_tensor("wi", [D, D], BF, kind="ExternalInput")
    wo_d = nc.dram_tensor("wo", [D, D], BF, kind="ExternalInput")
    bias_d = nc.dram_tensor("bias", [1, D], BF, kind="ExternalInput")
    dmask_d = nc.dram_tensor("dmask", [128, 512], BF, kind="ExternalInput")
    ident_d = nc.dram_tensor("ident", [128, 128], BF, kind="ExternalInput")
    out_d = nc.dram_tensor("out", [TOK, D], F32, kind="ExternalOutput")
    if debug:
        dbg = {
            "dbg_qT_my": nc.dram_tensor("dbg_qT_my", [128, 4, 8, 128], BF, kind="ExternalOutput"),
            "dbg_qT_all": nc.dram_tensor("dbg_qT_all", [128, 4, 4, 8, 128], BF, kind="ExternalOutput"),
            "dbg_qn_all": nc.dram_tensor("dbg_qn_all", [128, 16, D], BF, kind="ExternalOutput"),
            "dbg_E": nc.dram_tensor("dbg_E", [4, 128, S], BF, kind="ExternalOutput"),
            "dbg_ET": nc.dram_tensor("dbg_ET", [4, 128, 16, 128], BF, kind="ExternalOutput"),
            "dbg_hT": nc.dram_tensor("dbg_hT", [128, 8, TOK], BF, kind="ExternalOutput"),
            "dbg_hn": nc.dram_tensor("dbg_hn", [128, 4, D], BF, kind="ExternalOutput"),
            "dbg_weff": nc.dram_tensor("dbg_weff", [128, 8, D], BF, kind="ExternalOutput"),
        }

    # ---------------- collective buffers -------------------
    AR_G = [list(range(NCORES))]
    AG_G = [[0, 1, 2, 3], [4, 5, 6, 7]]
    qtg_in = [nc.dram_tensor(f"qtg_in{t}", [CHUNK], F8) for t in range(NT)]
    qtg_out = [
        nc.dram_tensor(
            f"qtg_out{t}", [4 * CHUNK], F8,
            addr_space=maybe_share_collective_output_space("AllGather", AG_G))
        for t in range(NT)
    ]
    wred_in = nc.dram_tensor("wred_in", [D, D], BF)
    wred_out = nc.dram_tensor(
        "wred_out", [D, D], BF,
        addr_space=maybe_share_collective_output_space("AllReduce", AR_G))

    with tile.TileContext(nc) as tc:
        with (
            tc.tile_pool(name="persist", bufs=1) as pp,
            tc.tile_pool(name="ps", bufs=4, space="PSUM") as psp,
            tc.tile_pool(name="pv", bufs=2, space="PSUM") as pvp,
            tc.tile_pool(name="tp", bufs=2, space="PSUM") as tpp,
            tc.tile_pool(name="E", bufs=4) as ep,
            tc.tile_pool(name="ET", bufs=3) as etp,
            tc.tile_pool(name="sq", bufs=1) as sqp,
            tc.tile_pool(name="outb", bufs=3) as outp,
            tc.tile_pool(name="mean", bufs=2) as mnp,
            tc.tile_pool(name="small", bufs=1) as smp,
        ):
            # persistent SBUF tensors
            wi_bf = pp.tile([128, 8, D], BF, tag="wi_bf")
            weff_bf = pp.tile([128, 8, D], BF, tag="weff_bf")
            xt_bf = pp.tile([128, 4, 8, 128], BF, tag="xt_bf")
            qT_my = pp.tile([128, 4, 8, 128], BF, tag="qT_my")
            qT_my8 = pp.tile([128, 4, 8, 128], F8, tag="qT_my8")
            # [bank, rank, dch, tok]: each gather chunk+rank lands as one
            # fully-contiguous load; scores read a bank via a strided AP
            qT_all = pp.tile([128, 4, 4, 8, 128], F8, tag="qT_all")
            qn_all = pp.tile([128, 16, D], F8, tag="qn_all")
            hT = pp.tile([128, 8, TOK], BF, tag="hT")
            h_raw = pp.tile([128, 4, D], BF, tag="h_raw")
            hn_sb = pp.tile([128, 4, D], BF, tag="hn_sb")
            hnT = pp.tile([128, 8, TOK], BF, tag="hnT")
            saT = pp.tile([128, 8, TOK], BF, tag="saT")
            dmask_sb = pp.tile([128, 512], BF, tag="dmask_sb")
            bias_sb = pp.tile([1, D], BF, tag="bias_sb")
            ident_sb = pp.tile([128, 128], BF, tag="ident_sb")
            ident8 = pp.tile([128, 128], F8, tag="ident8")

            ones1 = smp.tile([1, 128], BF, tag="ones1")
            acc = smp.tile([128, 16], F32, tag="acc")
            rinv = smp.tile([128, 4], F32, tag="rinv")
            st = smp.tile([128, 16], F32, tag="st")
            eps_ap = smp.tile([128, 1], F32, tag="eps_ap")
            ms = smp.tile([128, 4], F32, tag="ms")      # per-tile mean
            vs = smp.tile([128, 4], F32, tag="vs")      # per-tile var -> rstd
            sh = smp.tile([128, 4], F32, tag="sh")      # per-tile -mean*rstd
            nc.vector.memset(ones1[:], 1.0)
            nc.vector.memset(eps_ap[:], EPS)

            for rep in range(reps):
                # ---------- phase 0: input loads ----------
                # Big chunks (SP dma issue is ~1us serial each), ordered so
                # the first q matmuls (kd=0) unblock as early as possible.
                def ld_wi(k0, k1):
                    nc.sync.dma_start(
                        wi_bf[:, k0:k1, :],
                        wi_d[128 * k0:128 * k1, :]
                        .rearrange("(g p) d -> p g d", p=128))

                def ld_xt(t):
                    nc.sync.dma_start(
                        xt_bf[:, t, :, :],
                        xt_d[D * t:D * (t + 1), :]
                        .rearrange("(g p) t -> p g t", p=128))

                ld_wi(0, 2)
                ld_xt(0)
                ld_wi(2, 4)
                ld_wi(4, 6)
                ld_xt(1)
                ld_wi(6, 8)
                ld_xt(2)
                ld_xt(3)
                nc.scalar.dma_start(dmask_sb[:], dmask_d[:, :])
                nc.scalar.dma_start(bias_sb[:1, :], bias_d[:1, :])
                nc.scalar.dma_start(ident_sb[:], ident_d[:, :])
                if rep == 0:
                    nc.vector.tensor_copy(ident8[:], ident_sb[:])

                # ---------- phase 1: q^T = (x @ wi)^T, tile-chunked gather --
                # out chunk [dch 128, tok 128]: lhsT = wi (d-rows x d-out
                # chunk), rhs = xt (d-rows x tok) -> q^T directly.  Tiles are
                # processed in pairs with kd outermost so the first matmuls
                # only need the first wi/xt chunks (DMA-paced warmup).
                def finish_q_tile(t, qps):
                    for g in range(2):
                        nc.scalar.copy(
                            qT_my[:, t, 4 * g:4 * (g + 1), :],
                            qps[g][:].rearrange("p (m q) -> p m q", m=4))
                        nc.vector.tensor_copy(
                            qT_my8[:, t, 4 * g:4 * (g + 1), :],
                            qps[g][:].rearrange("p (m q) -> p m q", m=4))
                    # pack on the Act queue (follows the copies naturally);
                    # tile-major qT_my makes src and dst both contiguous
                    nc.scalar.dma_start(
                        qtg_in[t].ap().rearrange("(p m t) -> p m t", p=128, m=8),
                        qT_my8[:, t, :, :])
                    if sim_cc_as_dma:
                        for rr in range(4):
                            eng = nc.sync if rr < 2 else nc.gpsimd
                            eng.dma_start(
                                qtg_out[t][rr * CHUNK:(rr + 1) * CHUNK],
                                qtg_in[t][:])
                    else:
                        nc.gpsimd.collective_compute(
                            "AllGather", ALU.bypass, replica_groups=AG_G,
                            ins=[qtg_in[t].ap().opt()],
                            outs=[qtg_out[t].ap().opt()])
                    # chunk t -> qT_all bank t: per-rank fully-contiguous loads
                    for rr in range(4):
                        nc.sync.dma_start(
                            qT_all[:, t, rr, :, :],
                            qtg_out[t][rr * CHUNK:(rr + 1) * CHUNK]
                            .rearrange("(p m t) -> p m t", p=128, m=8))

                for t in range(NT):
                    qps = [psp.tile([128, 512], F32, tag="ps",
                                    name=f"qp{t}_{g}_{rep}") for g in range(2)]
                    for g in range(2):
                        for dl in range(4):
                            dch = 4 * g + dl
                            for kd in range(8):
                                nc.tensor.matmul(
                                    qps[g][:, 128 * dl:128 * (dl + 1)],
                                    wi_bf[:, kd, 128 * dch:128 * (dch + 1)],
                                    xt_bf[:, t, kd, :],
                                    start=(kd == 0), stop=(kd == 7))
                    finish_q_tile(t, qps)

                # ---------- W_eff: 8-core AllReduce straight off DRAM ------
                # Chunked transfers so the serial DMA engine never head-of-
                # line-blocks the latency-critical E/qn transposes behind a
                # multi-MB copy; wait-until keeps the chain out of the
                # attention-critical DMA window (weff is first needed by
                # out-proj at ~70us).
                with tc.tile_wait_until(0.035):
                    # collectives cannot read IO tensors: stage wo first
                    for ck in range(4):
                        nc.gpsimd.dma_start(
                            wred_in[256 * ck:256 * (ck + 1), :],
                            wo_d[256 * ck:256 * (ck + 1), :])
                    if sim_cc_as_dma:
                        for ck in range(4):
                            nc.gpsimd.dma_start(
                                wred_out[256 * ck:256 * (ck + 1), :],
                                wred_in[256 * ck:256 * (ck + 1), :])
                    else:
                        nc.gpsimd.collective_compute(
                            "AllReduce", ALU.add, replica_groups=AR_G,
                            ins=[wred_in.ap().opt()],
                            outs=[wred_out.ap().opt()])
                    for kh in range(2):
                        nc.gpsimd.dma_start(
                            weff_bf[:, 4 * kh:4 * (kh + 1), :],
                            wred_out[512 * kh:512 * (kh + 1), :]
                            .rearrange("(kt p) d -> p kt d", p=128))

                # ---------- attention + out-proj + LN, slot-pipelined ------
                E_tiles = {}
                ET_tiles = {}
                hp_tiles = {}

                def emit_sc_bank(s, n):
                    # one score PSUM bank: q rows of slot s x key bank n.
                    # Bank-major round order means bank n of EVERY slot runs
                    # as soon as the slot-n gather chunk lands.
                    E = E_tiles[s]
                    sc = psp.tile([128, 512], F32, tag="ps",
                                  name=f"sc{s}_{n}_{rep}")
                    # per-rank 128-wide chunks: chunk rr only needs gather
                    # load rr of bank n, so scores start on the first arrival
                    for rr in range(4):
                        for kp in range(4):
                            nc.tensor.matmul(
                                sc[:, 128 * rr:128 * (rr + 1)],
                                qT_my8[:, s, 2 * kp:2 * (kp + 1), :],
                                qT_all[:, n, rr, 2 * kp:2 * (kp + 1), :],
                                start=(kp == 0), stop=(kp == 3),
                                perf_mode=DR)
                    if n == s:
                        nc.vector.tensor_add(sc[:], sc[:], dmask_sb[:])
                    nc.scalar.activation(
                        E[:, 512 * n:512 * (n + 1)], sc[:], AF.Exp,
                        bias=0.0, scale=1.0 / 32.0,
                        accum_out=acc[:, 4 * s + n:4 * s + n + 1])

                def emit_softmax(s):
                    E = E_tiles[s]
                    W = 512 * (s + 1)
                    if s == 0:
                        nc.vector.reciprocal(rinv[:, 0:1], acc[:, 0:1])
                    else:
                        nc.vector.reduce_sum(rinv[:, s:s + 1],
                                             acc[:, 4 * s:4 * s + s + 1],
                                             axis=AX.X)
                        nc.vector.reciprocal(rinv[:, s:s + 1], rinv[:, s:s + 1])
                    # ET carries 64*P so small probabilities stay in fp8
                    # normal range; the PV residual divides the 64 back out
                    nc.vector.tensor_scalar_mul(rinv[:, s:s + 1],
                                                rinv[:, s:s + 1], 64.0)
                    nc.vector.tensor_scalar_mul(E[:, :W], E[:, :W],
                                                rinv[:, s:s + 1])
                    ET = etp.tile([128, 16, 128], F8, tag="ET",
                                  name=f"ET{s}_{rep}")
                    ET_tiles[s] = ET
                    # E -> E^T on the PE (identity transpose): DmaTranspose
                    # flips the DMA xbar mode and serializes against all
                    # in-flight copies, so keep it off the DMA entirely.
                    for g in range(s + 1):
                        tq = pvp.tile([128, 512], BF, tag="pv",
                                      name=f"te{s}_{g}_{rep}")
                        for k in range(4):
                            kt = 4 * g + k
                            nc.tensor.transpose(
                                tq[:, 128 * k:128 * (k + 1)],
                                E[:, 128 * kt:128 * (kt + 1)],
                                ident_sb[:])
                        eng = nc.scalar if g % 2 == 0 else nc.vector
                        if eng is nc.scalar:
                            eng.copy(ET[:, 4 * g:4 * (g + 1), :],
                                     tq[:].rearrange("p (k q) -> p k q", k=4))
                        else:
                            eng.tensor_copy(
                                ET[:, 4 * g:4 * (g + 1), :],
                                tq[:].rearrange("p (k q) -> p k q", k=4))
                    if debug:
                        nc.sync.dma_start(dbg["dbg_E"][s, :, 0:W], E[:, :W])
                        nc.sync.dma_start(
                            dbg["dbg_ET"][s, :, 0:4 * (s + 1), :],
                            ET[:, 0:4 * (s + 1), :])

                def emit_qnT(s):
                    # qn_all key tiles [4s, 4s+4) from gathered q^T via PE
                    # transpose (identity matmul, bf16 PSUM out) -- keeps the
                    # 4MB of transposes off the DMA queue.
                    for dch in range(8):
                        # fp8 PE transpose writes at element step 2 in PSUM
                        tp = tpp.tile([128, 1024], F8, tag="tp",
                                      name=f"tp{s}_{dch}_{rep}")
                        tpv = tp[:].rearrange("p (q two) -> p q two", two=2)
                        for k in range(4):
                            nc.tensor.transpose(
                                tpv[:, 128 * k:128 * (k + 1), 0],
                                qT_all[:, s, k, dch, :],
                                ident8[:])
                        eng = nc.scalar if dch % 2 == 0 else nc.vector
                        src_ap = tpv[:, :, 0].rearrange(
                            "p (k q) -> p k q", k=4)
                        if eng is nc.scalar:
                            eng.copy(
                                qn_all[:, 4 * s:4 * (s + 1),
                                       128 * dch:128 * (dch + 1)], src_ap)
                        else:
                            eng.tensor_copy(
                                qn_all[:, 4 * s:4 * (s + 1),
                                       128 * dch:128 * (dch + 1)], src_ap)

                def emit_pv(s):
                    ET = ET_tiles[s]
                    nkp = 2 * (s + 1)
                    for g in range(2):        # dch groups of 4
                        pv = pvp.tile([128, 512], F32, tag="pv",
                                      name=f"pv{s}_{g}_{rep}")
                        for dl in range(4):
                            m = 4 * g + dl
                            for kp in range(nkp):
                                nc.tensor.matmul(
                                    pv[:, 128 * dl:128 * (dl + 1)],
                                    qn_all[:, 2 * kp:2 * (kp + 1),
                                           128 * m:128 * (m + 1)],
                                    ET[:, 2 * kp:2 * (kp + 1), :],
                                    start=(kp == 0), stop=(kp == nkp - 1),
                                    perf_mode=DR)
                        for dl in range(4):
                            m = 4 * g + dl
                            nc.vector.scalar_tensor_tensor(
                                hT[:, m, 128 * s:128 * (s + 1)],
                                pv[:, 128 * dl:128 * (dl + 1)], 1.0 / 64.0,
                                qT_my[:, s, m, :],
                                op0=ALU.mult, op1=ALU.add)

                def emit_outproj(t):
                    hps = []
                    for hh in range(2):
                        hp = psp.tile([128, 512], F32, tag="ps",
                                      name=f"hp{t}_{hh}_{rep}")
                        for kd in range(8):
                            nc.tensor.matmul(
                                hp[:], hT[:, kd, 128 * t:128 * (t + 1)],
                                weff_bf[:, kd, 512 * hh:512 * (hh + 1)],
                                start=(kd == 0), stop=False)
                        nc.tensor.matmul(
                            hp[:], ones1[:1, :],
                            bias_sb[:1, 512 * hh:512 * (hh + 1)],
                            start=False, stop=True)
                        hps.append(hp)
                    # LN stats only (sums + sums of squares); rstd is batched
                    # across tiles to avoid act-func-set thrash.
                    c0 = 4 * t
                    for hh, hp in enumerate(hps):
                        nc.vector.reduce_sum(st[:, c0 + hh:c0 + hh + 1],
                                             hp[:], axis=AX.X)
                        sqs = sqp.tile([128, 512], BF, tag="sq",
                                       name=f"sq{t}_{hh}_{rep}")
                        nc.scalar.activation(
                            sqs[:], hp[:], AF.Square,
                            accum_out=st[:, c0 + 2 + hh:c0 + 3 + hh])
                        # park h in SBUF so the PSUM bank frees immediately
                        nc.vector.tensor_copy(
                            h_raw[:, t, 512 * hh:512 * (hh + 1)], hp[:])
                    nc.vector.tensor_scalar(
                        ms[:, t:t + 1], st[:, c0:c0 + 1],
                        st[:, c0 + 1:c0 + 2], 1.0 / D,
                        op0=ALU.add, op1=ALU.mult)
                    nc.vector.tensor_scalar(
                        vs[:, t:t + 1], st[:, c0 + 2:c0 + 3],
                        st[:, c0 + 3:c0 + 4], 1.0 / D,
                        op0=ALU.add, op1=ALU.mult)
                    # vs <- E[h^2] - mean^2  (sh column as scratch)
                    nc.vector.tensor_tensor(
                        sh[:, t:t + 1], ms[:, t:t + 1], ms[:, t:t + 1],
                        op=ALU.mult)
                    nc.vector.tensor_tensor(
                        vs[:, t:t + 1], vs[:, t:t + 1], sh[:, t:t + 1],
                        op=ALU.subtract)

                def emit_ln_pair(ta):
                    # rstd = exp(-0.5 * ln(var + eps)) on a [128,2] slice:
                    # one Ln + one Exp per pair; pair (0,1) overlaps op2/op3
                    # on PE so the FFN can start right after op3.
                    pr = slice(ta, ta + 2)
                    nc.scalar.activation(vs[:, pr], vs[:, pr], AF.Ln,
                                         bias=eps_ap[:, 0:1])
                    nc.scalar.activation(vs[:, pr], vs[:, pr], AF.Exp,
                                         bias=0.0, scale=-0.5)
                    nc.vector.tensor_tensor(sh[:, pr], ms[:, pr], vs[:, pr],
                                            op=ALU.mult)
                    for t in (ta, ta + 1):
                        for hh in range(2):
                            # (h * rstd) - (mean * rstd)
                            nc.vector.tensor_scalar(
                                hn_sb[:, t, 512 * hh:512 * (hh + 1)],
                                h_raw[:, t, 512 * hh:512 * (hh + 1)],
                                vs[:, t:t + 1], sh[:, t:t + 1],
                                op0=ALU.mult, op1=ALU.subtract)
                        # hn -> hn^T on PE (no DmaTranspose xbar switches)
                        for g in range(2):
                            tq = tpp.tile([128, 512], BF, tag="tp",
                                          name=f"th{t}_{g}_{rep}")
                            for k in range(4):
                                dch = 4 * g + k
                                nc.tensor.transpose(
                                    tq[:, 128 * k:128 * (k + 1)],
                                    hn_sb[:, t, 128 * dch:128 * (dch + 1)],
                                    ident_sb[:])
                            nc.scalar.copy(
                                hnT[:, 4 * g:4 * (g + 1),
                                    128 * t:128 * (t + 1)],
                                tq[:].rearrange("p (m q) -> p m q", m=4))

                for s in range(NT):
                    E_tiles[s] = ep.tile([128, S], BF, tag="E",
                                         name=f"E{s}_{rep}")
                for n in range(NT):
                    for s in range(n, NT):
                        emit_sc_bank(s, n)
                    emit_qnT(n)
                    emit_softmax(n)
                    if n >= 1:
                        emit_pv(n - 1)
                emit_outproj(0)
                emit_outproj(1)
                emit_ln_pair(0)
                emit_pv(3)
                emit_outproj(2)
                emit_outproj(3)
                emit_ln_pair(2)

                if debug:
                    nc.sync.dma_start(dbg["dbg_qT_my"][:], qT_my[:])
                    nc.sync.dma_start(dbg["dbg_qT_all"][:], qT_all[:])
                    nc.sync.dma_start(dbg["dbg_qn_all"][:], qn_all[:])
                    nc.sync.dma_start(dbg["dbg_hT"][:], hT[:])
                    nc.sync.dma_start(dbg["dbg_hn"][:], hn_sb[:])
                    nc.sync.dma_start(dbg["dbg_weff"][:], weff_bf[:])

                # ---------- FFN (token halves) ----------
                for th in range(2):
                    for mh in range(4):
                        f1 = psp.tile([128, 512], F32, tag="ps",
                                      name=f"f1_{rep}_{th}_{mh}")
                        for ml in range(2):
                            m = 2 * mh + ml
                            for kd in range(8):
                                nc.tensor.matmul(
                                    f1[:, 256 * ml:256 * (ml + 1)],
                                    wi_bf[:, kd, 128 * m:128 * (m + 1)],
                                    hnT[:, kd, 256 * th:256 * (th + 1)],
                                    start=(kd == 0), stop=(kd == 7))
                        nc.scalar.activation(
                            saT[:, 2 * mh:2 * (mh + 1),
                                256 * th:256 * (th + 1)],
                            f1[:].rearrange("p (m t) -> p m t", m=2),
                            AF.Silu)
                    for tt in (2 * th, 2 * th + 1):
                        for hh in range(2):
                            f2 = psp.tile([128, 512], F32, tag="ps",
                                          name=f"f2_{rep}_{tt}_{hh}")
                            for kd in range(8):
                                nc.tensor.matmul(
                                    f2[:], saT[:, kd, 128 * tt:128 * (tt + 1)],
                                    wi_bf[:, kd, 512 * hh:512 * (hh + 1)],
                                    start=(kd == 0), stop=(kd == 7))
                            ob = outp.tile([128, 512], F32, tag="outb",
                                           name=f"ob_{rep}_{tt}_{hh}")
                            nc.vector.tensor_copy(ob[:], f2[:])
                            nc.sync.dma_start(
                                out_d[128 * tt:128 * (tt + 1),
                                      512 * hh:512 * (hh + 1)], ob[:])

    nc.compile()
    return nc


def _get_nc(reps=1, sim_cc_as_dma=False, debug=False):
    key = ("nc", reps, sim_cc_as_dma, debug)
    if key not in _CACHE:
        _CACHE[key] = _build(reps, sim_cc_as_dma, debug)
    return _CACHE[key]


def make_in_maps(x, mask, wi, out_kernel, out_bias):
    """Host-side sharding: build the 8 per-core input dicts (bf16)."""
    import ml_dtypes

    bf = ml_dtypes.bfloat16
    x = np.asarray(x, dtype=np.float32)
    wi_b = np.ascontiguousarray(np.asarray(wi, np.float32).astype(bf))
    ok = np.asarray(out_kernel, np.float32)
    bias_b = np.ascontiguousarray(
        np.asarray(out_bias, np.float32).astype(bf).reshape(1, D))
    mask = np.asarray(mask).astype(bool)

    # structural requirements of the causal-padded schedule
    for t in range(NT):
        rows = slice(512 * t, 512 * t + 512)
        if t > 0:
            assert mask[rows, :512 * t].all(), \
                "mask must be all-visible below the diagonal banks"
        assert not mask[rows, 512 * (t + 1):].any(), \
            "mask must be all-hidden beyond the diagonal banks"
    ident = np.ascontiguousarray(np.eye(128, dtype=np.float32).astype(bf))

    in_maps = []
    for c in range(NCORES):
        b, r = divmod(c, 4)
        rows = np.concatenate(
            [np.arange(512 * t + 128 * r, 512 * t + 128 * (r + 1))
             for t in range(NT)])
        # tile-major: [4, D, 128] flattened -> each tile loads contiguously
        xt = np.ascontiguousarray(np.stack(
            [x[b, 512 * t + 128 * r:512 * t + 128 * (r + 1), :].T
             for t in range(NT)]).reshape(NT * D, 128).astype(bf))
        # diagonal-bank additive mask, identical across slots (assert)
        dms = []
        for t in range(NT):
            rt = slice(512 * t + 128 * r, 512 * t + 128 * (r + 1))
            ct = slice(512 * t, 512 * (t + 1))
            dms.append(np.where(mask[rt, ct], np.float32(0.0),
                                np.float32(MASK_NEG)))
        for t in range(1, NT):
            assert np.array_equal(dms[0], dms[t]), \
                "diagonal mask pattern must be slot-invariant"
        dmask = np.ascontiguousarray(dms[0].astype(bf))
        wo = np.ascontiguousarray(ok[D * c:D * (c + 1), :].astype(bf))
        in_maps.append({
            "xt": xt, "wi": wi_b, "wo": wo, "bias": bias_b, "dmask": dmask,
            "ident": ident,
        })
    return in_maps


def assemble_output(results):
    out = np.empty((B, S, D), dtype=np.float32)
    for c in range(NCORES):
        b, r = divmod(c, 4)
        res = results[c]["out"]
        for t in range(NT):
            out[b, 512 * t + 128 * r:512 * t + 128 * (r + 1), :] = \
                res[128 * t:128 * (t + 1), :]
    return out


def kernel(x, mask, wi, out_kernel, out_bias, n_heads):
    from concourse.bass_utils import run_bass_kernel_spmd

    assert int(np.asarray(n_heads)) == H
    nc = _get_nc()
    in_maps = make_in_maps(x, mask, wi, out_kernel, out_bias)
    res = run_bass_kernel_spmd(nc, in_maps, core_ids=list(range(NCORES)))
    return assemble_output(res.results)


if __name__ == "__main__":
    # quick self-check against the reference if available
    sys.path.insert(0, "/root/problem")
    import reference

    inputs = {k: np.asarray(v) for k, v in reference.setup_inputs().items()}
    exp = np.asarray(reference.reference(**reference.setup_inputs()))
    act = kernel(**inputs)
    err = np.linalg.norm(act - exp) / np.linalg.norm(exp)
    print("Relative error:", err)
